# revision 76
# baseline (speedup 1.0000x reference)
"""CrossBatchAttention kernel for 8 Trainium2 NeuronCores.

Strategy: tensor-parallel over heads. 16 heads / 8 cores = 2 heads per core.
Each core computes (all matmul operands fp8e4, fp32 PSUM accumulation):
  - v   = (hidden @ Wv.T) in [b, e] layout: fp8 DoubleRow over kt pairs,
    kt-outer in two waves of 8 bank-aligned PSUM accumulators, overlapping
    the streaming-in of hidden.T
  - qT/kT = (W @ hidden.T) in [e, b] layout, fp8 DoubleRow over kt pairs
  - attention in 4 (head, q-half) sub-phases over kt PAIRS, one pair deep
    software-pipelined: scoresT[k, q] = kT @ qT; exp(score-3) split between
    ACT (native Exp -> fp8) and DVE (Schraudolph bit-trick: affine to uint8
    bits read back as fp8e4); diagonal zeroed multiplicatively; the fp8 ex
    pair feeds DoubleRow ctx matmuls and DoubleRow ones-matmuls that
    accumulate row sums in a psB-resident tile. The ctx accumulator is
    evicted RAW (bf16) and normalized in SBUF next sub-phase with
    rb = 4/rowsum broadcast via a SWDGE partition-broadcast DMA, giving
    ctxT = 64x natural scale in fp8 with no PSUM-slot pressure on the
    scores/exp rotation.
  - crossT[o, b] = wo-pair @ ctxT-pair (DoubleRow, wo stationary shared
    across b-halves), evictions alternating DVE/ACT, output DMA spread
    over three engine queues. Output is transposed [H, B].
Host: sums the 8 partial crossT, out = hidden + sigmoid(s)/1024 * cross.T.

The residual path keeps hidden in fp32 exactly; cross contributes only
~0.1% of output magnitude, so the fp8/Schraudolph error (~2% of cross) is
~2e-5 end-to-end against a 2e-2 tolerance.
"""

import numpy as np
import ml_dtypes

B = 2048
H = 2048
NH = 16
HD = 128
NCORES = 8
HL = NH // NCORES          # heads per core = 2
E = HL * HD                # local projection width = 256
P = 128
KT = H // P                # 16 contraction tiles over hidden dim
BT = B // P                # 16 row tiles

_BF16 = ml_dtypes.bfloat16

_compiled = None


def _build():
    import concourse.bass as bass  # noqa: F401
    import concourse.tile as tile
    from concourse import bacc, mybir

    bf = mybir.dt.bfloat16
    f8 = mybir.dt.float8e4
    f32 = mybir.dt.float32
    Exp = mybir.ActivationFunctionType.Exp
    mult = mybir.AluOpType.mult
    # fp8 range scaling: weights x16 on host, exp() rescales scores back
    expscale = float(1.0 / (256.0 * 128.0 ** 0.5))

    nc = bacc.Bacc(
        "TRN2",
        target_bir_lowering=False,
        debug=False,
        enable_asserts=False,
        num_devices=NCORES,
    )

    # inputs are pre-packed on the host to match SBUF layout exactly, so every
    # DMA row is a large contiguous span (descriptor-efficient)
    hT_d = nc.dram_tensor("hT", [P, KT * B], f8, kind="ExternalInput").ap()
    wqT_d = nc.dram_tensor("wqT", [P, KT * E], f8, kind="ExternalInput").ap()
    wkT_d = nc.dram_tensor("wkT", [P, KT * E], f8, kind="ExternalInput").ap()
    wvT_d = nc.dram_tensor("wvT", [P, KT * E], f8, kind="ExternalInput").ap()
    woT_d = nc.dram_tensor("woT", [P, HL * H], f8, kind="ExternalInput").ap()
    antiI_d = nc.dram_tensor("antiI", [P, P], bf, kind="ExternalInput").ap()
    # transposed output crossT[o, b] in fp8 (values ~N(0,9), well within
    # fp8e4 range; quantization ~2% of cross -> ~2e-5 end-to-end); host
    # transposes back (free on CPU)
    out_d = nc.dram_tensor("out", [H, B], f8, kind="ExternalOutput").ap()

    with tile.TileContext(nc) as tc:
        with (
            tc.tile_pool(name="const", bufs=1) as constp,
            tc.tile_pool(name="work", bufs=1) as workp,
            tc.tile_pool(name="stream", bufs=8) as streamp,
            tc.tile_pool(name="psA", bufs=2, space="PSUM") as psA,
            tc.tile_pool(name="psB", bufs=2, space="PSUM") as psB,
        ):
            # ---------------- constants ----------------
            antiI = constp.tile([P, P], bf)
            # 0.25 folds the ctxT scale into the row-sums: rrow = 4/sum, so
            # ctxT = 64 * natural-scale ctx (max|ctx| ~ 0.7 -> ~45 << 240 fp8)
            # ones pair for the DoubleRow row-sum matmuls: [p, 2, 1] with
            # 16-byte pair stride (DR AP constraint)
            ones32 = constp.tile([P, 32], f8)
            nc.gpsimd.memset(ones32[:], 0.25)
            ones3 = ones32.rearrange("p (t u) -> p t u", t=2)
            zbias = constp.tile([P, 1], f32)
            nc.gpsimd.memset(zbias[:], 0.0)
            # exp argument shift: ex = exp(score - 3) keeps fp8e4 values in
            # [0, ~165] (max score ~7.1); the shift cancels in normalization
            bias3 = constp.tile([P, 1], f32)
            nc.gpsimd.memset(bias3[:], -3.0)
            # preload the exp table set (~2.7us) while DMA streams in
            dummy = constp.tile([1, 1], f32)
            nc.scalar.activation(dummy[:], zbias[0:1, 0:1], Exp, bias=zbias[0:1, 0:1], scale=1.0)

            # ------- input DMA, ordered to match consumption order -------
            # few, large descriptors: each dma_start costs ~620ns of issue
            # time on its engine queue, so the baseline's 38 descriptors
            # serialized ~9us before the first matmul could start.
            hT_sb = constp.tile([P, KT * B], f8)
            wq_sb = constp.tile([P, KT * E], f8)
            wk_sb = constp.tile([P, KT * E], f8)
            wv_sb = constp.tile([P, KT * E], f8)
            wo_sb = constp.tile([P, HL * H], f8)

            hT3d = hT_d.rearrange("p (t b) -> p t b", t=KT)
            hT3s = hT_sb.rearrange("p (t b) -> p t b", t=KT)
            # strict consumption order (v consumes kt PAIRS), small leading
            # chunks so the first v matmuls start right after the ~7.5us
            # framework preamble
            nc.sync.dma_start(hT3s[:, 0:2, 0:1024], hT3d[:, 0:2, 0:1024])
            nc.sync.dma_start(wv_sb[:, :2 * E], wvT_d[:, :2 * E])
            nc.sync.dma_start(hT3s[:, 2:4, 0:1024], hT3d[:, 2:4, 0:1024])
            nc.sync.dma_start(wv_sb[:, 2 * E:], wvT_d[:, 2 * E:])
            # v consumes c2=0 (wave 0) kt-ascending, then c2=1 (wave 1)
            for g in range(3):
                nc.sync.dma_start(
                    hT3s[:, 4 * g + 4:4 * g + 8, 0:1024],
                    hT3d[:, 4 * g + 4:4 * g + 8, 0:1024],
                )
            for g in range(2):
                nc.sync.dma_start(
                    hT3s[:, 8 * g:8 * g + 8, 1024:2048],
                    hT3d[:, 8 * g:8 * g + 8, 1024:2048],
                )
            # later-needed weights at the end of the queue (consumed ~31us+)
            nc.sync.dma_start(wq_sb[:], wqT_d[:])
            nc.sync.dma_start(wk_sb[:], wkT_d[:])
            nc.scalar.dma_start(wo_sb[:], woT_d[:])
            nc.scalar.dma_start(antiI[:], antiI_d[:])

            qT_sb = workp.tile([P, HL * B], f8)   # [d, b] per head at h*B
            kT_sb = workp.tile([P, HL * B], f8)
            v_sb = workp.tile([P, BT * E], f8)    # [b%128, bt*E + e], 16x nat
            ctxT_sb = workp.tile([P, HL * B], f8)  # 64x natural scale

            # ------- v projection: fp8 DoubleRow over kt pairs, kt-outer, ----
            # 2 waves of 8 bank-aligned accumulators. (An interleaved
            # v+qk variant measured slower: the projection window is paced
            # by the input DMA stream, not the PE.)
            hT3 = hT_sb.rearrange("p (t b) -> p t b", t=KT)
            wv3 = wv_sb.rearrange("p (t e) -> p t e", t=KT)
            wq3 = wq_sb.rearrange("p (t e) -> p t e", t=KT)
            wk3 = wk_sb.rearrange("p (t e) -> p t e", t=KT)
            DR = mybir.MatmulPerfMode.DoubleRow
            for wave in range(2):
                vt0 = psA.tile([P, 1024], f32, tag="A", name=f"vt0_{wave}")
                vt1 = psA.tile([P, 1024], f32, tag="A", name=f"vt1_{wave}")
                vt2 = psB.tile([P, 1024], f32, tag="B", name=f"vt2_{wave}")
                vt3 = psB.tile([P, 1024], f32, tag="B", name=f"vt3_{wave}")
                vtiles = [vt0, vt1, vt2, vt3]

                def vacc(j):
                    # accumulator j -> own PSUM bank: slot j//2, col (j%2)*512
                    return vtiles[j // 2][:, (j % 2) * 512: (j % 2) * 512 + 256]

                for kt2 in range(KT // 2):
                    for j in range(8):
                        bt = wave * 8 + j
                        nc.tensor.matmul(
                            vacc(j),
                            lhsT=hT3[:, 2 * kt2:2 * kt2 + 2, bt * P:(bt + 1) * P],
                            rhs=wv3[:, 2 * kt2:2 * kt2 + 2, :],
                            start=(kt2 == 0),
                            stop=(kt2 == KT // 2 - 1),
                            perf_mode=DR,
                        )
                for j in range(8):
                    bt = wave * 8 + j
                    nc.any.tensor_copy(v_sb[:, bt * E:(bt + 1) * E], vacc(j))

            # ---------------- q/k projections (out = [e, b]) ----------------
            # fp8 DoubleRow: 2 hidden-dim rows per PE cell.
            for nm, dst, w3 in (("q", qT_sb, wq3), ("k", kT_sb, wk3)):
                for et in range(HL):
                    for bh in range(B // 1024):
                        psp = psA.tile([P, 1024], f32, tag="A",
                                       name=f"psp_{nm}_{et}_{bh}")
                        for kt2 in range(KT // 2):
                            for b2 in range(2):
                                bc = bh * 2 + b2
                                nc.tensor.matmul(
                                    psp[:, b2 * 512:(b2 + 1) * 512],
                                    lhsT=w3[:, 2 * kt2:2 * kt2 + 2, et * P:(et + 1) * P],
                                    rhs=hT3[:, 2 * kt2:2 * kt2 + 2, bc * 512:(bc + 1) * 512],
                                    start=(kt2 == 0),
                                    stop=(kt2 == KT // 2 - 1),
                                    perf_mode=DR,
                                )
                        nc.any.tensor_copy(
                            dst[:, et * B + bh * 1024: et * B + (bh + 1) * 1024], psp[:])

            # ------------- output projection helper (interleavable) -----------
            # transposed form: crossT[o, b] = sum_e wo[e, o] * ctxT[e, b].
            # fp8 DoubleRow packs both head tiles (et) into one matmul, and
            # the stationary operand (wo pair for o-tile ot) is shared by
            # both b-half matmuls -> LDWEIGHTS amortizes.
            ctxT3 = ctxT_sb.rearrange("p (h b) -> p h b", h=HL)
            wo3 = wo_sb.rearrange("p (h o) -> p h o", h=HL)
            out_engines = [nc.sync, nc.scalar, nc.gpsimd]

            def emit_cross(ot, bh, idx, split=True, use_b=False):
                # alternate pools to deepen the psx rotation: the two DR
                # matmuls are faster than one eviction, so rotation depth
                # limits throughput here
                pool, tg = (psB, "B") if use_b else (psA, "A")
                psx = pool.tile([P, 1024], f32, tag=tg, name=f"psx_{ot}_{bh}")
                for b2 in range(2):
                    nc.tensor.matmul(
                        psx[:, b2 * 512:(b2 + 1) * 512],
                        lhsT=wo3[:, :, ot * P:(ot + 1) * P],
                        rhs=ctxT3[:, :, bh * 1024 + b2 * 512: bh * 1024 + (b2 + 1) * 512],
                        start=True,
                        stop=True,
                        perf_mode=DR,
                    )
                xo = streamp.tile([P, 1024], f8, tag="xo", name=f"xo_{ot}_{bh}")
                if split and idx % 2 == 1:
                    nc.scalar.copy(xo[:], psx[:])
                else:
                    nc.vector.tensor_copy(xo[:], psx[:])
                out_engines[idx % 3].dma_start(
                    out_d[ot * P:(ot + 1) * P, bh * 1024:(bh + 1) * 1024], xo[:]
                )

            # ---------------- attention: 4 (head, q-half) sub-phases ----------
            # kt-PAIR structured: scores -> exp (split across ACT and a DVE
            # Schraudolph bit-trick) -> fp8 ex pair consumed by DoubleRow ctx
            # and DoubleRow ones row-sum matmuls. Row sums accumulate in a
            # psB-resident tile (the slot freed by evicting ctxps RAW and
            # normalizing later in SBUF). No DVE accumulator at all.
            u8 = mybir.dt.uint8
            v4 = v_sb.rearrange("p (t e) -> p t e", t=BT)
            LOG2E = 1.4426950408889634
            # Schraudolph fp8e4: bits ~= 8*(log2(v) + 7 + 0.0573)
            A_DVE = float(8.0 * LOG2E * expscale)
            B_DVE = float(8.0 * (7.0 + 0.0573 - 3.0 * LOG2E))
            # which kt's exp runs on DVE instead of ACT (load balance)
            DVE_KTS = (2, 5, 8, 11)
            pending = [None]

            def norm_pending():
                st = pending[0]
                if st is None:
                    return
                h, qh = st["h"], st["qh"]
                nc.vector.tensor_tensor(
                    ctxT_sb[:, h * B + qh * 1024: h * B + (qh + 1) * 1024],
                    st["ctxraw"][:],
                    st["rb"][:],
                    op=mult,
                )
                pending[0] = None

            for h in range(HL):
                for qh in range(2):
                    last = (h == HL - 1 and qh == 1)
                    ctxps = psB.tile([P, 1024], f32, tag="B", name=f"ctxps_{h}_{qh}")
                    epB = psB.tile([P, 1024], f32, tag="B", name=f"epB_{h}_{qh}")

                    def consume_pair(pr, ex2):
                        # ctx + row-sum DoubleRow matmuls over an exp pair
                        ex3 = ex2.rearrange("p (t q) -> p t q", t=2)
                        for q2 in range(2):
                            nc.tensor.matmul(
                                ctxps[:, q2 * 512:(q2 + 1) * 512],
                                lhsT=v4[:, 2 * pr:2 * pr + 2, h * P:(h + 1) * P],
                                rhs=ex3[:, :, q2 * 512:(q2 + 1) * 512],
                                start=(pr == 0),
                                stop=(pr == KT // 2 - 1),
                                perf_mode=DR,
                            )
                        for q2 in range(2):
                            nc.tensor.matmul(
                                epB[0:1, q2 * 512:(q2 + 1) * 512],
                                lhsT=ones3[:, :, 0:1],
                                rhs=ex3[:, :, q2 * 512:(q2 + 1) * 512],
                                start=(pr == 0),
                                stop=(pr == KT // 2 - 1),
                                perf_mode=DR,
                            )

                    # software-pipelined one pair deep: pair p's consumers
                    # are emitted after pair p+1's scores, so the PE never
                    # waits in-FIFO on the exp latency
                    prev_pair = [None]
                    for pr in range(KT // 2):
                        # previous sub-phase's normalization once its
                        # broadcast rb has landed (~2us into this sub-phase)
                        if pr == 1:
                            norm_pending()
                        ex2 = streamp.tile([P, 2048], f8, tag="exp", bufs=12,
                                           name=f"ex2_{h}_{qh}_{pr}")
                        for k2 in range(2):
                            kt = 2 * pr + k2
                            pss = psA.tile([P, 1024], f32, tag="A",
                                           name=f"pss_{h}_{qh}_{kt}")
                            for q2 in range(2):
                                qc = qh * 2 + q2
                                nc.tensor.matmul(
                                    pss[:, q2 * 512:(q2 + 1) * 512],
                                    lhsT=kT_sb[:, h * B + kt * P: h * B + (kt + 1) * P],
                                    rhs=qT_sb[:, h * B + qc * 512: h * B + (qc + 1) * 512],
                                    start=True,
                                    stop=True,
                                )
                            exh = ex2[:, k2 * 1024:(k2 + 1) * 1024]
                            if kt in DVE_KTS:
                                # exp(score-3) as fp8 bits: affine + value-
                                # convert to uint8, read back as fp8e4
                                nc.vector.tensor_scalar(
                                    exh.bitcast(u8), pss[:], A_DVE, B_DVE,
                                    op0=mult, op1=mybir.AluOpType.add,
                                )
                            else:
                                nc.scalar.activation(exh, pss[:], Exp,
                                                     bias=bias3[:, 0:1],
                                                     scale=expscale)
                            if kt // 8 == qh:
                                off = k2 * 1024 + kt * P - qh * 1024
                                nc.vector.tensor_tensor(
                                    ex2[:, off:off + P], ex2[:, off:off + P],
                                    antiI[:], op=mult,
                                )
                        if prev_pair[0] is not None:
                            consume_pair(*prev_pair[0])
                        prev_pair[0] = (pr, ex2)
                    consume_pair(*prev_pair[0])
                    # sub-phase epilogue: reciprocal frees epB, raw-evict
                    # frees ctxps; the norm runs next sub-phase once the
                    # broadcast lands
                    rrow = workp.tile([1, 1024], f32, tag="rrow", bufs=2,
                                      name=f"rrow_{h}_{qh}")
                    nc.vector.reciprocal_approx_fast(rrow[:], epB[0:1, :])
                    ctxraw = workp.tile([P, 1024], bf, tag="craw", bufs=2,
                                        name=f"craw_{h}_{qh}")
                    nc.vector.tensor_copy(ctxraw[:], ctxps[:])
                    rb = workp.tile([P, 1024], f32, tag="rb", bufs=2,
                                    name=f"rb_{h}_{qh}")
                    nc.gpsimd.partition_broadcast(rb[:], rrow[0:1, :])
                    pending[0] = {"h": h, "qh": qh, "ctxraw": ctxraw, "rb": rb}

            # ---------------- output projection ----------------
            # (ot, 0) units first: they only need q columns 0-1023, already
            # normalized. The final sub-phase's norm is emitted a few units
            # in so it doesn't head-of-line-block their DVE evictions while
            # its broadcast rb is still in flight.
            norm_pending()
            order = [(ot, 0) for ot in range(BT)] + [(ot, 1) for ot in range(BT)]
            for idx, (ot, bh) in enumerate(order):
                emit_cross(ot, bh, idx, use_b=(idx % 2 == 1))

    nc.compile()
    return nc


def _get_compiled():
    global _compiled
    if _compiled is None:
        _compiled = _build()
    return _compiled


def _numpy_reference(hidden_states, attention_mask, Wq, Wk, Wv, Wo, scale_param):
    hs = np.asarray(hidden_states, np.float64)
    q = (hs @ np.asarray(Wq, np.float64).T).reshape(B, NH, HD).transpose(1, 0, 2)
    k = (hs @ np.asarray(Wk, np.float64).T).reshape(B, NH, HD).transpose(1, 0, 2)
    v = (hs @ np.asarray(Wv, np.float64).T).reshape(B, NH, HD).transpose(1, 0, 2)
    scores = np.einsum("hqd,hkd->hqk", q, k) / (HD ** 0.5)
    eye = np.eye(B, dtype=bool)
    scores = np.where(eye[None, :, :], -np.inf, scores)
    mask = np.asarray(attention_mask, bool)
    scores = np.where(mask[None, None, :], scores, -np.inf)
    m = scores.max(axis=-1, keepdims=True)
    m = np.where(np.isfinite(m), m, 0.0)
    e = np.exp(scores - m)
    s = e.sum(axis=-1, keepdims=True)
    attn = np.where(s > 0, e / np.maximum(s, 1e-300), 0.0)
    ctx = np.einsum("hqk,hkd->hqd", attn, v)
    ctx = ctx.transpose(1, 0, 2).reshape(B, H)
    cross = ctx @ np.asarray(Wo, np.float64).T
    scale = 1.0 / (1.0 + np.exp(-float(np.asarray(scale_param).reshape(-1)[0])))
    return (hs + scale * cross).astype(np.float32)


LAST_RESULTS = None


def kernel(hidden_states, attention_mask, Wq, Wk, Wv, Wo, scale_param):
    hs = np.asarray(hidden_states, np.float32)
    mask = np.asarray(attention_mask, bool)
    if not mask.all():
        return _numpy_reference(hidden_states, mask, Wq, Wk, Wv, Wo, scale_param)

    from concourse import bass_utils

    nc = _get_compiled()

    def pack(a):
        # [T*128, W] -> [128, T*W]: row p holds tile-t's row p for every t,
        # matching the SBUF destination layout exactly (contiguous DMA rows)
        t = a.shape[0] // P
        return np.ascontiguousarray(
            a.reshape(t, P, a.shape[1]).transpose(1, 0, 2).reshape(P, t * a.shape[1])
        )

    from concourse import mybir as _mybir
    _F8 = _mybir.dt.np(_mybir.dt.float8e4)

    hT = pack(hs.T.astype(_F8))
    antiI = (1.0 - np.eye(P, dtype=np.float32)).astype(_BF16)
    Wq = np.asarray(Wq, np.float32)
    Wk = np.asarray(Wk, np.float32)
    Wv = np.asarray(Wv, np.float32)
    Wo = np.asarray(Wo, np.float32)

    in_maps = []
    for c in range(NCORES):
        rs = slice(c * E, (c + 1) * E)
        in_maps.append({
            "hT": hT,
            "wqT": pack((Wq[rs, :].T * np.float32(16.0)).astype(_F8)),
            "wkT": pack((Wk[rs, :].T * np.float32(16.0)).astype(_F8)),
            "wvT": pack((Wv[rs, :].T * np.float32(16.0)).astype(_F8)),
            "woT": pack((Wo[:, rs].T * np.float32(16.0)).astype(_F8)),
            "antiI": antiI,
        })

    import os
    res = bass_utils.run_bass_kernel_spmd(
        nc, in_maps, core_ids=list(range(NCORES)),
        trace=bool(os.environ.get("KERNEL_TRACE")),
    )
    global LAST_RESULTS
    LAST_RESULTS = res

    cross = np.zeros((H, B), np.float32)
    for r in res.results:
        cross += np.asarray(r["out"], np.float32)
    # device emits crossT[o, b] at 1024x natural scale (ctxT 64x, wo 16x)
    cross = cross.T * np.float32(1.0 / 1024.0)
    scale = np.float32(1.0 / (1.0 + np.exp(-float(np.asarray(scale_param).reshape(-1)[0]))))
    return (hs + scale * cross).astype(np.float32)



# revision 77
# speedup vs baseline: 1.0202x; 1.0202x over previous
"""CrossBatchAttention kernel for 8 Trainium2 NeuronCores.

Strategy: tensor-parallel over heads. 16 heads / 8 cores = 2 heads per core.
Each core computes (all matmul operands fp8e4, fp32 PSUM accumulation):
  - v   = (hidden @ Wv.T) in [b, e] layout: fp8 DoubleRow over kt pairs,
    kt-outer in two waves of 8 bank-aligned PSUM accumulators, overlapping
    the streaming-in of hidden.T
  - qT/kT = (W @ hidden.T) in [e, b] layout, fp8 DoubleRow over kt pairs
  - attention in 4 (head, q-half) sub-phases over kt PAIRS, one pair deep
    software-pipelined: scoresT[k, q] = kT @ qT; exp(score-3) split between
    ACT (native Exp -> fp8) and DVE (Schraudolph bit-trick: affine to uint8
    bits read back as fp8e4); diagonal zeroed multiplicatively; the fp8 ex
    pair feeds DoubleRow ctx matmuls and DoubleRow ones-matmuls that
    accumulate row sums in a psB-resident tile. The ctx accumulator is
    evicted RAW (bf16) and normalized in SBUF next sub-phase with
    rb = 4/rowsum broadcast via a SWDGE partition-broadcast DMA, giving
    ctxT = 64x natural scale in fp8 with no PSUM-slot pressure on the
    scores/exp rotation.
  - crossT[o, b] = wo-pair @ ctxT-pair (DoubleRow, wo stationary shared
    across b-halves), evictions alternating DVE/ACT, output DMA spread
    over three engine queues. Output is transposed [H, B].
Host: sums the 8 partial crossT, out = hidden + sigmoid(s)/1024 * cross.T.

The residual path keeps hidden in fp32 exactly; cross contributes only
~0.1% of output magnitude, so the fp8/Schraudolph error (~2% of cross) is
~2e-5 end-to-end against a 2e-2 tolerance.
"""

import numpy as np
import ml_dtypes

B = 2048
H = 2048
NH = 16
HD = 128
NCORES = 8
HL = NH // NCORES          # heads per core = 2
E = HL * HD                # local projection width = 256
P = 128
KT = H // P                # 16 contraction tiles over hidden dim
BT = B // P                # 16 row tiles

_BF16 = ml_dtypes.bfloat16

_compiled = None


def _build():
    import concourse.bass as bass  # noqa: F401
    import concourse.tile as tile
    from concourse import bacc, mybir

    bf = mybir.dt.bfloat16
    f8 = mybir.dt.float8e4
    f32 = mybir.dt.float32
    Exp = mybir.ActivationFunctionType.Exp
    mult = mybir.AluOpType.mult
    # fp8 range scaling: weights x16 on host, exp() rescales scores back
    expscale = float(1.0 / (256.0 * 128.0 ** 0.5))

    nc = bacc.Bacc(
        "TRN2",
        target_bir_lowering=False,
        debug=False,
        enable_asserts=False,
        num_devices=NCORES,
    )

    # inputs are pre-packed on the host to match SBUF layout exactly, so every
    # DMA row is a large contiguous span (descriptor-efficient)
    hT_d = nc.dram_tensor("hT", [P, KT * B], f8, kind="ExternalInput").ap()
    wqT_d = nc.dram_tensor("wqT", [P, KT * E], f8, kind="ExternalInput").ap()
    wkT_d = nc.dram_tensor("wkT", [P, KT * E], f8, kind="ExternalInput").ap()
    wvT_d = nc.dram_tensor("wvT", [P, KT * E], f8, kind="ExternalInput").ap()
    woT_d = nc.dram_tensor("woT", [P, HL * H], f8, kind="ExternalInput").ap()
    antiI_d = nc.dram_tensor("antiI", [P, P], bf, kind="ExternalInput").ap()
    # transposed output crossT[o, b] in fp8 (values ~N(0,9), well within
    # fp8e4 range; quantization ~2% of cross -> ~2e-5 end-to-end); host
    # transposes back (free on CPU)
    out_d = nc.dram_tensor("out", [H, B], f8, kind="ExternalOutput").ap()

    with tile.TileContext(nc) as tc:
        with (
            tc.tile_pool(name="const", bufs=1) as constp,
            tc.tile_pool(name="work", bufs=1) as workp,
            tc.tile_pool(name="stream", bufs=8) as streamp,
            tc.tile_pool(name="psA", bufs=2, space="PSUM") as psA,
            tc.tile_pool(name="psB", bufs=2, space="PSUM") as psB,
        ):
            # ---------------- constants ----------------
            antiI = constp.tile([P, P], bf)
            # 0.25 folds the ctxT scale into the row-sums: rrow = 4/sum, so
            # ctxT = 64 * natural-scale ctx (max|ctx| ~ 0.7 -> ~45 << 240 fp8)
            # ones pair for the DoubleRow row-sum matmuls: [p, 2, 1] with
            # 16-byte pair stride (DR AP constraint)
            ones32 = constp.tile([P, 32], f8)
            nc.gpsimd.memset(ones32[:], 0.25)
            ones3 = ones32.rearrange("p (t u) -> p t u", t=2)
            zbias = constp.tile([P, 1], f32)
            nc.gpsimd.memset(zbias[:], 0.0)
            # exp argument shift: ex = exp(score - 3) keeps fp8e4 values in
            # [0, ~165] (max score ~7.1); the shift cancels in normalization
            bias3 = constp.tile([P, 1], f32)
            nc.gpsimd.memset(bias3[:], -3.0)
            # preload the exp table set (~2.7us) while DMA streams in
            dummy = constp.tile([1, 1], f32)
            nc.scalar.activation(dummy[:], zbias[0:1, 0:1], Exp, bias=zbias[0:1, 0:1], scale=1.0)

            # ------- input DMA, ordered to match consumption order -------
            # few, large descriptors: each dma_start costs ~620ns of issue
            # time on its engine queue, so the baseline's 38 descriptors
            # serialized ~9us before the first matmul could start.
            hT_sb = constp.tile([P, KT * B], f8)
            wq_sb = constp.tile([P, KT * E], f8)
            wk_sb = constp.tile([P, KT * E], f8)
            wv_sb = constp.tile([P, KT * E], f8)
            wo_sb = constp.tile([P, HL * H], f8)

            hT3d = hT_d.rearrange("p (t b) -> p t b", t=KT)
            hT3s = hT_sb.rearrange("p (t b) -> p t b", t=KT)
            # strict consumption order (v consumes kt PAIRS), small leading
            # chunks so the first v matmuls start right after the ~7.5us
            # framework preamble
            nc.sync.dma_start(hT3s[:, 0:2, 0:1024], hT3d[:, 0:2, 0:1024])
            nc.sync.dma_start(wv_sb[:, :2 * E], wvT_d[:, :2 * E])
            nc.sync.dma_start(hT3s[:, 2:4, 0:1024], hT3d[:, 2:4, 0:1024])
            nc.sync.dma_start(wv_sb[:, 2 * E:], wvT_d[:, 2 * E:])
            # v consumes c2=0 (wave 0) kt-ascending, then c2=1 (wave 1)
            for g in range(3):
                nc.sync.dma_start(
                    hT3s[:, 4 * g + 4:4 * g + 8, 0:1024],
                    hT3d[:, 4 * g + 4:4 * g + 8, 0:1024],
                )
            for g in range(2):
                nc.sync.dma_start(
                    hT3s[:, 8 * g:8 * g + 8, 1024:2048],
                    hT3d[:, 8 * g:8 * g + 8, 1024:2048],
                )
            # later-needed weights at the end of the queue (consumed ~31us+)
            nc.sync.dma_start(wq_sb[:], wqT_d[:])
            nc.sync.dma_start(wk_sb[:], wkT_d[:])
            nc.scalar.dma_start(wo_sb[:], woT_d[:])
            nc.scalar.dma_start(antiI[:], antiI_d[:])

            qT_sb = workp.tile([P, HL * B], f8)   # [d, b] per head at h*B
            kT_sb = workp.tile([P, HL * B], f8)
            v_sb = workp.tile([P, BT * E], f8)    # [b%128, bt*E + e], 16x nat
            ctxT_sb = workp.tile([P, HL * B], f8)  # 64x natural scale

            # ------- v projection: fp8 DoubleRow over kt pairs, kt-outer, ----
            # 2 waves of 8 bank-aligned accumulators. (An interleaved
            # v+qk variant measured slower: the projection window is paced
            # by the input DMA stream, not the PE.)
            hT3 = hT_sb.rearrange("p (t b) -> p t b", t=KT)
            wv3 = wv_sb.rearrange("p (t e) -> p t e", t=KT)
            wq3 = wq_sb.rearrange("p (t e) -> p t e", t=KT)
            wk3 = wk_sb.rearrange("p (t e) -> p t e", t=KT)
            DR = mybir.MatmulPerfMode.DoubleRow
            for wave in range(2):
                vt0 = psA.tile([P, 1024], f32, tag="A", name=f"vt0_{wave}")
                vt1 = psA.tile([P, 1024], f32, tag="A", name=f"vt1_{wave}")
                vt2 = psB.tile([P, 1024], f32, tag="B", name=f"vt2_{wave}")
                vt3 = psB.tile([P, 1024], f32, tag="B", name=f"vt3_{wave}")
                vtiles = [vt0, vt1, vt2, vt3]

                def vacc(j):
                    # accumulator j -> own PSUM bank: slot j//2, col (j%2)*512
                    return vtiles[j // 2][:, (j % 2) * 512: (j % 2) * 512 + 256]

                for kt2 in range(KT // 2):
                    for j in range(8):
                        bt = wave * 8 + j
                        nc.tensor.matmul(
                            vacc(j),
                            lhsT=hT3[:, 2 * kt2:2 * kt2 + 2, bt * P:(bt + 1) * P],
                            rhs=wv3[:, 2 * kt2:2 * kt2 + 2, :],
                            start=(kt2 == 0),
                            stop=(kt2 == KT // 2 - 1),
                            perf_mode=DR,
                        )
                for j in range(8):
                    bt = wave * 8 + j
                    nc.any.tensor_copy(v_sb[:, bt * E:(bt + 1) * E], vacc(j))

            # ---------------- q/k projections (out = [e, b]) ----------------
            # fp8 DoubleRow: 2 hidden-dim rows per PE cell.
            for nm, dst, w3 in (("q", qT_sb, wq3), ("k", kT_sb, wk3)):
                for et in range(HL):
                    for bh in range(B // 1024):
                        psp = psA.tile([P, 1024], f32, tag="A",
                                       name=f"psp_{nm}_{et}_{bh}")
                        for kt2 in range(KT // 2):
                            for b2 in range(2):
                                bc = bh * 2 + b2
                                nc.tensor.matmul(
                                    psp[:, b2 * 512:(b2 + 1) * 512],
                                    lhsT=w3[:, 2 * kt2:2 * kt2 + 2, et * P:(et + 1) * P],
                                    rhs=hT3[:, 2 * kt2:2 * kt2 + 2, bc * 512:(bc + 1) * 512],
                                    start=(kt2 == 0),
                                    stop=(kt2 == KT // 2 - 1),
                                    perf_mode=DR,
                                )
                        nc.any.tensor_copy(
                            dst[:, et * B + bh * 1024: et * B + (bh + 1) * 1024], psp[:])

            # ------------- output projection helper (interleavable) -----------
            # transposed form: crossT[o, b] = sum_e wo[e, o] * ctxT[e, b].
            # fp8 DoubleRow packs both head tiles (et) into one matmul, and
            # the stationary operand (wo pair for o-tile ot) is shared by
            # both b-half matmuls -> LDWEIGHTS amortizes.
            ctxT3 = ctxT_sb.rearrange("p (h b) -> p h b", h=HL)
            wo3 = wo_sb.rearrange("p (h o) -> p h o", h=HL)
            out_engines = [nc.sync, nc.scalar, nc.gpsimd]

            def emit_cross(ot, bh, idx, split=True, use_b=False):
                # alternate pools to deepen the psx rotation: the two DR
                # matmuls are faster than one eviction, so rotation depth
                # limits throughput here
                pool, tg = (psB, "B") if use_b else (psA, "A")
                psx = pool.tile([P, 1024], f32, tag=tg, name=f"psx_{ot}_{bh}")
                for b2 in range(2):
                    nc.tensor.matmul(
                        psx[:, b2 * 512:(b2 + 1) * 512],
                        lhsT=wo3[:, :, ot * P:(ot + 1) * P],
                        rhs=ctxT3[:, :, bh * 1024 + b2 * 512: bh * 1024 + (b2 + 1) * 512],
                        start=True,
                        stop=True,
                        perf_mode=DR,
                    )
                xo = streamp.tile([P, 1024], f8, tag="xo", name=f"xo_{ot}_{bh}")
                if split and idx % 2 == 1:
                    nc.scalar.copy(xo[:], psx[:])
                else:
                    nc.vector.tensor_copy(xo[:], psx[:])
                out_engines[idx % 3].dma_start(
                    out_d[ot * P:(ot + 1) * P, bh * 1024:(bh + 1) * 1024], xo[:]
                )

            # ---------------- attention: 4 (head, q-half) sub-phases ----------
            # kt-PAIR structured: scores -> exp (split across ACT and a DVE
            # Schraudolph bit-trick) -> fp8 ex pair consumed by DoubleRow ctx
            # and DoubleRow ones row-sum matmuls. Row sums accumulate in a
            # psB-resident tile (the slot freed by evicting ctxps RAW and
            # normalizing later in SBUF). No DVE accumulator at all.
            u8 = mybir.dt.uint8
            v4 = v_sb.rearrange("p (t e) -> p t e", t=BT)
            LOG2E = 1.4426950408889634
            # Schraudolph fp8e4: bits ~= 8*(log2(v) + 7 + 0.0573)
            A_DVE = float(8.0 * LOG2E * expscale)
            B_DVE = float(8.0 * (7.0 + 0.0573 - 3.0 * LOG2E))
            # which kt's exp runs on DVE instead of ACT (load balance)
            DVE_KTS = (2, 5, 8, 11, 14)
            pending = [None]

            def norm_pending():
                st = pending[0]
                if st is None:
                    return
                h, qh = st["h"], st["qh"]
                nc.vector.tensor_tensor(
                    ctxT_sb[:, h * B + qh * 1024: h * B + (qh + 1) * 1024],
                    st["ctxraw"][:],
                    st["rb"][:],
                    op=mult,
                )
                pending[0] = None

            for h in range(HL):
                for qh in range(2):
                    last = (h == HL - 1 and qh == 1)
                    ctxps = psB.tile([P, 1024], f32, tag="B", name=f"ctxps_{h}_{qh}")
                    epB = psB.tile([P, 1024], f32, tag="B", name=f"epB_{h}_{qh}")

                    def consume_pair(pr, ex2):
                        # ctx + row-sum DoubleRow matmuls over an exp pair
                        ex3 = ex2.rearrange("p (t q) -> p t q", t=2)
                        for q2 in range(2):
                            nc.tensor.matmul(
                                ctxps[:, q2 * 512:(q2 + 1) * 512],
                                lhsT=v4[:, 2 * pr:2 * pr + 2, h * P:(h + 1) * P],
                                rhs=ex3[:, :, q2 * 512:(q2 + 1) * 512],
                                start=(pr == 0),
                                stop=(pr == KT // 2 - 1),
                                perf_mode=DR,
                            )
                        for q2 in range(2):
                            nc.tensor.matmul(
                                epB[0:1, q2 * 512:(q2 + 1) * 512],
                                lhsT=ones3[:, :, 0:1],
                                rhs=ex3[:, :, q2 * 512:(q2 + 1) * 512],
                                start=(pr == 0),
                                stop=(pr == KT // 2 - 1),
                                perf_mode=DR,
                            )

                    # software-pipelined one pair deep: pair p's consumers
                    # are emitted after pair p+1's scores, so the PE never
                    # waits in-FIFO on the exp latency
                    prev_pair = [None]
                    for pr in range(KT // 2):
                        # previous sub-phase's normalization once its
                        # broadcast rb has landed (~2us into this sub-phase)
                        if pr == 1:
                            norm_pending()
                        ex2 = streamp.tile([P, 2048], f8, tag="exp", bufs=12,
                                           name=f"ex2_{h}_{qh}_{pr}")
                        for k2 in range(2):
                            kt = 2 * pr + k2
                            pss = psA.tile([P, 1024], f32, tag="A",
                                           name=f"pss_{h}_{qh}_{kt}")
                            for q2 in range(2):
                                qc = qh * 2 + q2
                                nc.tensor.matmul(
                                    pss[:, q2 * 512:(q2 + 1) * 512],
                                    lhsT=kT_sb[:, h * B + kt * P: h * B + (kt + 1) * P],
                                    rhs=qT_sb[:, h * B + qc * 512: h * B + (qc + 1) * 512],
                                    start=True,
                                    stop=True,
                                )
                            exh = ex2[:, k2 * 1024:(k2 + 1) * 1024]
                            if kt in DVE_KTS:
                                # exp(score-3) as fp8 bits: affine + value-
                                # convert to uint8, read back as fp8e4
                                nc.vector.tensor_scalar(
                                    exh.bitcast(u8), pss[:], A_DVE, B_DVE,
                                    op0=mult, op1=mybir.AluOpType.add,
                                )
                            else:
                                nc.scalar.activation(exh, pss[:], Exp,
                                                     bias=bias3[:, 0:1],
                                                     scale=expscale)
                            if kt // 8 == qh:
                                off = k2 * 1024 + kt * P - qh * 1024
                                nc.vector.tensor_tensor(
                                    ex2[:, off:off + P], ex2[:, off:off + P],
                                    antiI[:], op=mult,
                                )
                        if prev_pair[0] is not None:
                            consume_pair(*prev_pair[0])
                        prev_pair[0] = (pr, ex2)
                    consume_pair(*prev_pair[0])
                    # sub-phase epilogue: reciprocal frees epB, raw-evict
                    # frees ctxps; the norm runs next sub-phase once the
                    # broadcast lands
                    rrow = workp.tile([1, 1024], f32, tag="rrow", bufs=2,
                                      name=f"rrow_{h}_{qh}")
                    nc.vector.reciprocal_approx_fast(rrow[:], epB[0:1, :])
                    ctxraw = workp.tile([P, 1024], bf, tag="craw", bufs=2,
                                        name=f"craw_{h}_{qh}")
                    nc.vector.tensor_copy(ctxraw[:], ctxps[:])
                    rb = workp.tile([P, 1024], f32, tag="rb", bufs=2,
                                    name=f"rb_{h}_{qh}")
                    nc.gpsimd.partition_broadcast(rb[:], rrow[0:1, :])
                    pending[0] = {"h": h, "qh": qh, "ctxraw": ctxraw, "rb": rb}

            # ---------------- output projection ----------------
            # (ot, 0) units first: they only need q columns 0-1023, already
            # normalized. The final sub-phase's norm is emitted a few units
            # in so it doesn't head-of-line-block their DVE evictions while
            # its broadcast rb is still in flight.
            norm_pending()
            order = [(ot, 0) for ot in range(BT)] + [(ot, 1) for ot in range(BT)]
            for idx, (ot, bh) in enumerate(order):
                emit_cross(ot, bh, idx, use_b=(idx % 2 == 1))

    nc.compile()
    return nc


def _get_compiled():
    global _compiled
    if _compiled is None:
        _compiled = _build()
    return _compiled


def _numpy_reference(hidden_states, attention_mask, Wq, Wk, Wv, Wo, scale_param):
    hs = np.asarray(hidden_states, np.float64)
    q = (hs @ np.asarray(Wq, np.float64).T).reshape(B, NH, HD).transpose(1, 0, 2)
    k = (hs @ np.asarray(Wk, np.float64).T).reshape(B, NH, HD).transpose(1, 0, 2)
    v = (hs @ np.asarray(Wv, np.float64).T).reshape(B, NH, HD).transpose(1, 0, 2)
    scores = np.einsum("hqd,hkd->hqk", q, k) / (HD ** 0.5)
    eye = np.eye(B, dtype=bool)
    scores = np.where(eye[None, :, :], -np.inf, scores)
    mask = np.asarray(attention_mask, bool)
    scores = np.where(mask[None, None, :], scores, -np.inf)
    m = scores.max(axis=-1, keepdims=True)
    m = np.where(np.isfinite(m), m, 0.0)
    e = np.exp(scores - m)
    s = e.sum(axis=-1, keepdims=True)
    attn = np.where(s > 0, e / np.maximum(s, 1e-300), 0.0)
    ctx = np.einsum("hqk,hkd->hqd", attn, v)
    ctx = ctx.transpose(1, 0, 2).reshape(B, H)
    cross = ctx @ np.asarray(Wo, np.float64).T
    scale = 1.0 / (1.0 + np.exp(-float(np.asarray(scale_param).reshape(-1)[0])))
    return (hs + scale * cross).astype(np.float32)


LAST_RESULTS = None


def kernel(hidden_states, attention_mask, Wq, Wk, Wv, Wo, scale_param):
    hs = np.asarray(hidden_states, np.float32)
    mask = np.asarray(attention_mask, bool)
    if not mask.all():
        return _numpy_reference(hidden_states, mask, Wq, Wk, Wv, Wo, scale_param)

    from concourse import bass_utils

    nc = _get_compiled()

    def pack(a):
        # [T*128, W] -> [128, T*W]: row p holds tile-t's row p for every t,
        # matching the SBUF destination layout exactly (contiguous DMA rows)
        t = a.shape[0] // P
        return np.ascontiguousarray(
            a.reshape(t, P, a.shape[1]).transpose(1, 0, 2).reshape(P, t * a.shape[1])
        )

    from concourse import mybir as _mybir
    _F8 = _mybir.dt.np(_mybir.dt.float8e4)

    hT = pack(hs.T.astype(_F8))
    antiI = (1.0 - np.eye(P, dtype=np.float32)).astype(_BF16)
    Wq = np.asarray(Wq, np.float32)
    Wk = np.asarray(Wk, np.float32)
    Wv = np.asarray(Wv, np.float32)
    Wo = np.asarray(Wo, np.float32)

    in_maps = []
    for c in range(NCORES):
        rs = slice(c * E, (c + 1) * E)
        in_maps.append({
            "hT": hT,
            "wqT": pack((Wq[rs, :].T * np.float32(16.0)).astype(_F8)),
            "wkT": pack((Wk[rs, :].T * np.float32(16.0)).astype(_F8)),
            "wvT": pack((Wv[rs, :].T * np.float32(16.0)).astype(_F8)),
            "woT": pack((Wo[:, rs].T * np.float32(16.0)).astype(_F8)),
            "antiI": antiI,
        })

    import os
    res = bass_utils.run_bass_kernel_spmd(
        nc, in_maps, core_ids=list(range(NCORES)),
        trace=bool(os.environ.get("KERNEL_TRACE")),
    )
    global LAST_RESULTS
    LAST_RESULTS = res

    cross = np.zeros((H, B), np.float32)
    for r in res.results:
        cross += np.asarray(r["out"], np.float32)
    # device emits crossT[o, b] at 1024x natural scale (ctxT 64x, wo 16x)
    cross = cross.T * np.float32(1.0 / 1024.0)
    scale = np.float32(1.0 / (1.0 + np.exp(-float(np.asarray(scale_param).reshape(-1)[0]))))
    return (hs + scale * cross).astype(np.float32)



# revision 78
# speedup vs baseline: 1.2090x; 1.1850x over previous
"""CrossBatchAttention kernel for 8 Trainium2 NeuronCores.

Strategy: tensor-parallel over heads. 16 heads / 8 cores = 2 heads per core.
Each core computes (all matmul operands fp8e4, fp32 PSUM accumulation):
  - v   = (hidden @ Wv.T) in [b, e] layout: fp8 DoubleRow over kt pairs,
    kt-outer in two waves of 8 bank-aligned PSUM accumulators, overlapping
    the streaming-in of hidden.T
  - qT/kT = (W @ hidden.T) in [e, b] layout, fp8 DoubleRow over kt pairs
  - attention in 4 (head, q-half) sub-phases over kt PAIRS, one pair deep
    software-pipelined: scoresT[k, q] = kT @ qT; exp(score-3) split between
    ACT (native Exp -> fp8) and DVE (Schraudolph bit-trick: affine to uint8
    bits read back as fp8e4); diagonal zeroed multiplicatively; the fp8 ex
    pair feeds DoubleRow ctx matmuls and DoubleRow ones-matmuls that
    accumulate row sums in a psB-resident tile. The ctx accumulator is
    evicted RAW (bf16) and normalized in SBUF next sub-phase with
    rb = 4/rowsum broadcast via a SWDGE partition-broadcast DMA, giving
    ctxT = 64x natural scale in fp8 with no PSUM-slot pressure on the
    scores/exp rotation.
  - crossT[o, b] = wo-pair @ ctxT-pair (DoubleRow, wo stationary shared
    across b-halves), evictions alternating DVE/ACT, output DMA spread
    over three engine queues. Output is transposed [H, B].
Host: sums the 8 partial crossT, out = hidden + sigmoid(s)/1024 * cross.T.

The residual path keeps hidden in fp32 exactly; cross contributes only
~0.1% of output magnitude, so the fp8/Schraudolph error (~2% of cross) is
~2e-5 end-to-end against a 2e-2 tolerance.
"""

import numpy as np
import ml_dtypes

B = 2048
H = 2048
NH = 16
HD = 128
NCORES = 8
HL = NH // NCORES          # heads per core = 2
E = HL * HD                # local projection width = 256
P = 128
KT = H // P                # 16 contraction tiles over hidden dim
BT = B // P                # 16 row tiles

_BF16 = ml_dtypes.bfloat16

_compiled = None


def _build():
    import concourse.bass as bass  # noqa: F401
    import concourse.tile as tile
    from concourse import bacc, mybir

    bf = mybir.dt.bfloat16
    f8 = mybir.dt.float8e4
    f32 = mybir.dt.float32
    Exp = mybir.ActivationFunctionType.Exp
    mult = mybir.AluOpType.mult
    # fp8 range scaling: weights x16 on host, exp() rescales scores back
    expscale = float(1.0 / (256.0 * 128.0 ** 0.5))

    nc = bacc.Bacc(
        "TRN2",
        target_bir_lowering=False,
        debug=False,
        enable_asserts=False,
        num_devices=NCORES,
    )

    # inputs are pre-packed on the host to match SBUF layout exactly, so every
    # DMA row is a large contiguous span (descriptor-efficient)
    hT_d = nc.dram_tensor("hT", [P, KT * B], f8, kind="ExternalInput").ap()
    wqT_d = nc.dram_tensor("wqT", [P, KT * E], f8, kind="ExternalInput").ap()
    wkT_d = nc.dram_tensor("wkT", [P, KT * E], f8, kind="ExternalInput").ap()
    wvT_d = nc.dram_tensor("wvT", [P, KT * E], f8, kind="ExternalInput").ap()
    woT_d = nc.dram_tensor("woT", [P, HL * H], f8, kind="ExternalInput").ap()
    antiI_d = nc.dram_tensor("antiI", [P, P], bf, kind="ExternalInput").ap()
    # transposed output crossT[o, b] in fp8 (values ~N(0,9), well within
    # fp8e4 range; quantization ~2% of cross -> ~2e-5 end-to-end); host
    # transposes back (free on CPU)
    out_d = nc.dram_tensor("out", [H, B], f8, kind="ExternalOutput").ap()

    with tile.TileContext(nc) as tc:
        with (
            tc.tile_pool(name="const", bufs=1) as constp,
            tc.tile_pool(name="work", bufs=1) as workp,
            tc.tile_pool(name="stream", bufs=8) as streamp,
            tc.tile_pool(name="psA", bufs=2, space="PSUM") as psA,
            tc.tile_pool(name="psB", bufs=2, space="PSUM") as psB,
        ):
            # ---------------- constants ----------------
            antiI = constp.tile([P, P], bf)
            # 0.25 folds the ctxT scale into the row-sums: rrow = 4/sum, so
            # ctxT = 64 * natural-scale ctx (max|ctx| ~ 0.7 -> ~45 << 240 fp8)
            # ones pair for the DoubleRow row-sum matmuls: [p, 2, 1] with
            # 16-byte pair stride (DR AP constraint)
            ones32 = constp.tile([P, 32], f8)
            nc.gpsimd.memset(ones32[:], 0.25)
            ones3 = ones32.rearrange("p (t u) -> p t u", t=2)
            zbias = constp.tile([P, 1], f32)
            nc.gpsimd.memset(zbias[:], 0.0)
            # exp argument shift: ex = exp(score - 3) keeps fp8e4 values in
            # [0, ~165] (max score ~7.1); the shift cancels in normalization
            bias3 = constp.tile([P, 1], f32)
            nc.gpsimd.memset(bias3[:], -3.0)
            # preload the exp table set (~2.7us) while DMA streams in
            dummy = constp.tile([1, 1], f32)
            nc.scalar.activation(dummy[:], zbias[0:1, 0:1], Exp, bias=zbias[0:1, 0:1], scale=1.0)

            # ------- input DMA, ordered to match consumption order -------
            # few, large descriptors: each dma_start costs ~620ns of issue
            # time on its engine queue, so the baseline's 38 descriptors
            # serialized ~9us before the first matmul could start.
            hT_sb = constp.tile([P, KT * B], f8)
            wq_sb = constp.tile([P, KT * E], f8)
            wk_sb = constp.tile([P, KT * E], f8)
            wv_sb = constp.tile([P, KT * E], f8)
            wo_sb = constp.tile([P, HL * H], f8)

            hT3d = hT_d.rearrange("p (t b) -> p t b", t=KT)
            hT3s = hT_sb.rearrange("p (t b) -> p t b", t=KT)
            # strict consumption order (v consumes kt PAIRS), small leading
            # chunks so the first v matmuls start right after the ~7.5us
            # framework preamble
            nc.sync.dma_start(hT3s[:, 0:2, 0:1024], hT3d[:, 0:2, 0:1024])
            nc.sync.dma_start(wv_sb[:, :2 * E], wvT_d[:, :2 * E])
            nc.sync.dma_start(hT3s[:, 2:4, 0:1024], hT3d[:, 2:4, 0:1024])
            nc.sync.dma_start(wv_sb[:, 2 * E:], wvT_d[:, 2 * E:])
            # v consumes c2=0 (wave 0) kt-ascending, then c2=1 (wave 1)
            for g in range(3):
                nc.sync.dma_start(
                    hT3s[:, 4 * g + 4:4 * g + 8, 0:1024],
                    hT3d[:, 4 * g + 4:4 * g + 8, 0:1024],
                )
            for g in range(2):
                nc.sync.dma_start(
                    hT3s[:, 8 * g:8 * g + 8, 1024:2048],
                    hT3d[:, 8 * g:8 * g + 8, 1024:2048],
                )
            # later-needed weights at the end of the queue (consumed ~31us+)
            nc.sync.dma_start(wq_sb[:], wqT_d[:])
            nc.sync.dma_start(wk_sb[:], wkT_d[:])
            nc.scalar.dma_start(wo_sb[:], woT_d[:])
            nc.scalar.dma_start(antiI[:], antiI_d[:])

            qT_sb = workp.tile([P, HL * B], f8)   # [d, b] per head at h*B
            kT_sb = workp.tile([P, HL * B], f8)
            v_sb = workp.tile([P, BT * E], f8)    # [b%128, bt*E + e], 16x nat
            ctxT_sb = workp.tile([P, HL * B], f8)  # 64x natural scale

            # ------- v projection: fp8 DoubleRow over kt pairs, kt-outer, ----
            # 2 waves of 8 bank-aligned accumulators. (An interleaved
            # v+qk variant measured slower: the projection window is paced
            # by the input DMA stream, not the PE.)
            hT3 = hT_sb.rearrange("p (t b) -> p t b", t=KT)
            wv3 = wv_sb.rearrange("p (t e) -> p t e", t=KT)
            wq3 = wq_sb.rearrange("p (t e) -> p t e", t=KT)
            wk3 = wk_sb.rearrange("p (t e) -> p t e", t=KT)
            DR = mybir.MatmulPerfMode.DoubleRow
            for wave in range(2):
                vt0 = psA.tile([P, 1024], f32, tag="A", name=f"vt0_{wave}")
                vt1 = psA.tile([P, 1024], f32, tag="A", name=f"vt1_{wave}")
                vt2 = psB.tile([P, 1024], f32, tag="B", name=f"vt2_{wave}")
                vt3 = psB.tile([P, 1024], f32, tag="B", name=f"vt3_{wave}")
                vtiles = [vt0, vt1, vt2, vt3]

                def vacc(j):
                    # accumulator j -> own PSUM bank: slot j//2, col (j%2)*512
                    return vtiles[j // 2][:, (j % 2) * 512: (j % 2) * 512 + 256]

                for kt2 in range(KT // 2):
                    for j in range(8):
                        bt = wave * 8 + j
                        nc.tensor.matmul(
                            vacc(j),
                            lhsT=hT3[:, 2 * kt2:2 * kt2 + 2, bt * P:(bt + 1) * P],
                            rhs=wv3[:, 2 * kt2:2 * kt2 + 2, :],
                            start=(kt2 == 0),
                            stop=(kt2 == KT // 2 - 1),
                            perf_mode=DR,
                        )
                for j in range(8):
                    bt = wave * 8 + j
                    nc.any.tensor_copy(v_sb[:, bt * E:(bt + 1) * E], vacc(j))

            # ---------------- q/k projections (out = [e, b]) ----------------
            # fp8 DoubleRow: 2 hidden-dim rows per PE cell.
            for nm, dst, w3 in (("q", qT_sb, wq3), ("k", kT_sb, wk3)):
                for et in range(HL):
                    for bh in range(B // 1024):
                        psp = psA.tile([P, 1024], f32, tag="A",
                                       name=f"psp_{nm}_{et}_{bh}")
                        for kt2 in range(KT // 2):
                            for b2 in range(2):
                                bc = bh * 2 + b2
                                nc.tensor.matmul(
                                    psp[:, b2 * 512:(b2 + 1) * 512],
                                    lhsT=w3[:, 2 * kt2:2 * kt2 + 2, et * P:(et + 1) * P],
                                    rhs=hT3[:, 2 * kt2:2 * kt2 + 2, bc * 512:(bc + 1) * 512],
                                    start=(kt2 == 0),
                                    stop=(kt2 == KT // 2 - 1),
                                    perf_mode=DR,
                                )
                        nc.any.tensor_copy(
                            dst[:, et * B + bh * 1024: et * B + (bh + 1) * 1024], psp[:])

            # ------------- output projection helper (interleavable) -----------
            # transposed form: crossT[o, b] = sum_e wo[e, o] * ctxT[e, b].
            # fp8 DoubleRow packs both head tiles (et) into one matmul, and
            # the stationary operand (wo pair for o-tile ot) is shared by
            # both b-half matmuls -> LDWEIGHTS amortizes.
            ctxT3 = ctxT_sb.rearrange("p (h b) -> p h b", h=HL)
            wo3 = wo_sb.rearrange("p (h o) -> p h o", h=HL)
            out_engines = [nc.sync, nc.scalar, nc.gpsimd]

            def emit_cross(ot, bh, idx, split=True, use_b=False):
                # alternate pools to deepen the psx rotation: the two DR
                # matmuls are faster than one eviction, so rotation depth
                # limits throughput here
                pool, tg = (psB, "B") if use_b else (psA, "A")
                psx = pool.tile([P, 1024], f32, tag=tg, name=f"psx_{ot}_{bh}")
                for b2 in range(2):
                    nc.tensor.matmul(
                        psx[:, b2 * 512:(b2 + 1) * 512],
                        lhsT=wo3[:, :, ot * P:(ot + 1) * P],
                        rhs=ctxT3[:, :, bh * 1024 + b2 * 512: bh * 1024 + (b2 + 1) * 512],
                        start=True,
                        stop=True,
                        perf_mode=DR,
                    )
                xo = streamp.tile([P, 1024], f8, tag="xo", name=f"xo_{ot}_{bh}")
                # 14 ACT / 18 DVE: the scalar queue also carries a third of
                # the output DMA issues, so it gets fewer evictions
                if split and idx % 2 == 1 and idx % 16 != 15:
                    nc.scalar.copy(xo[:], psx[:])
                else:
                    nc.vector.tensor_copy(xo[:], psx[:])
                out_engines[idx % 3].dma_start(
                    out_d[ot * P:(ot + 1) * P, bh * 1024:(bh + 1) * 1024], xo[:]
                )

            # ---------------- attention: 4 (head, q-half) sub-phases ----------
            # kt-PAIR structured: scores -> exp (split across ACT and a DVE
            # Schraudolph bit-trick) -> fp8 ex pair consumed by DoubleRow ctx
            # and DoubleRow ones row-sum matmuls. Row sums accumulate in a
            # psB-resident tile (the slot freed by evicting ctxps RAW and
            # normalizing later in SBUF). No DVE accumulator at all.
            u8 = mybir.dt.uint8
            v4 = v_sb.rearrange("p (t e) -> p t e", t=BT)
            LOG2E = 1.4426950408889634
            # Schraudolph fp8e4: bits ~= 8*(log2(v) + 7 + 0.0573)
            A_DVE = float(8.0 * LOG2E * expscale)
            B_DVE = float(8.0 * (7.0 + 0.0573 - 3.0 * LOG2E))
            # which kt's exp runs on DVE instead of ACT (load balance)
            DVE_KTS = (2, 5, 8, 11, 14)
            pending = [None]

            def norm_pending():
                st = pending[0]
                if st is None:
                    return
                h, qh = st["h"], st["qh"]
                nc.vector.tensor_tensor(
                    ctxT_sb[:, h * B + qh * 1024: h * B + (qh + 1) * 1024],
                    st["ctxraw"][:],
                    st["rb"][:],
                    op=mult,
                )
                pending[0] = None

            for h in range(HL):
                for qh in range(2):
                    last = (h == HL - 1 and qh == 1)
                    ctxps = psB.tile([P, 1024], f32, tag="B", name=f"ctxps_{h}_{qh}")
                    epB = psB.tile([P, 1024], f32, tag="B", name=f"epB_{h}_{qh}")

                    def consume_pair(pr, ex2):
                        # ctx + row-sum DoubleRow matmuls over an exp pair
                        ex3 = ex2.rearrange("p (t q) -> p t q", t=2)
                        for q2 in range(2):
                            nc.tensor.matmul(
                                ctxps[:, q2 * 512:(q2 + 1) * 512],
                                lhsT=v4[:, 2 * pr:2 * pr + 2, h * P:(h + 1) * P],
                                rhs=ex3[:, :, q2 * 512:(q2 + 1) * 512],
                                start=(pr == 0),
                                stop=(pr == KT // 2 - 1),
                                perf_mode=DR,
                            )
                        for q2 in range(2):
                            nc.tensor.matmul(
                                epB[0:1, q2 * 512:(q2 + 1) * 512],
                                lhsT=ones3[:, :, 0:1],
                                rhs=ex3[:, :, q2 * 512:(q2 + 1) * 512],
                                start=(pr == 0),
                                stop=(pr == KT // 2 - 1),
                                perf_mode=DR,
                            )

                    # software-pipelined one pair deep: pair p's consumers
                    # are emitted after pair p+1's scores, so the PE never
                    # waits in-FIFO on the exp latency
                    prev_pair = [None]
                    for pr in range(KT // 2):
                        # previous sub-phase's normalization once its
                        # broadcast rb has landed (~2us into this sub-phase)
                        if pr == 1:
                            norm_pending()
                        ex2 = streamp.tile([P, 2048], f8, tag="exp", bufs=12,
                                           name=f"ex2_{h}_{qh}_{pr}")
                        for k2 in range(2):
                            kt = 2 * pr + k2
                            pss = psA.tile([P, 1024], f32, tag="A",
                                           name=f"pss_{h}_{qh}_{kt}")
                            for q2 in range(2):
                                qc = qh * 2 + q2
                                nc.tensor.matmul(
                                    pss[:, q2 * 512:(q2 + 1) * 512],
                                    lhsT=kT_sb[:, h * B + kt * P: h * B + (kt + 1) * P],
                                    rhs=qT_sb[:, h * B + qc * 512: h * B + (qc + 1) * 512],
                                    start=True,
                                    stop=True,
                                )
                            exh = ex2[:, k2 * 1024:(k2 + 1) * 1024]
                            if kt in DVE_KTS:
                                # exp(score-3) as fp8 bits: affine + value-
                                # convert to uint8, read back as fp8e4
                                nc.vector.tensor_scalar(
                                    exh.bitcast(u8), pss[:], A_DVE, B_DVE,
                                    op0=mult, op1=mybir.AluOpType.add,
                                )
                            else:
                                nc.scalar.activation(exh, pss[:], Exp,
                                                     bias=bias3[:, 0:1],
                                                     scale=expscale)
                            if kt // 8 == qh:
                                off = k2 * 1024 + kt * P - qh * 1024
                                nc.vector.tensor_tensor(
                                    ex2[:, off:off + P], ex2[:, off:off + P],
                                    antiI[:], op=mult,
                                )
                        if prev_pair[0] is not None:
                            consume_pair(*prev_pair[0])
                        prev_pair[0] = (pr, ex2)
                    consume_pair(*prev_pair[0])
                    # sub-phase epilogue: reciprocal frees epB, raw-evict
                    # frees ctxps; the norm runs next sub-phase once the
                    # broadcast lands
                    rrow = workp.tile([1, 1024], f32, tag="rrow", bufs=2,
                                      name=f"rrow_{h}_{qh}")
                    nc.vector.reciprocal_approx_fast(rrow[:], epB[0:1, :])
                    ctxraw = workp.tile([P, 1024], bf, tag="craw", bufs=2,
                                        name=f"craw_{h}_{qh}")
                    nc.vector.tensor_copy(ctxraw[:], ctxps[:])
                    rb = workp.tile([P, 1024], f32, tag="rb", bufs=2,
                                    name=f"rb_{h}_{qh}")
                    nc.gpsimd.partition_broadcast(rb[:], rrow[0:1, :])
                    pending[0] = {"h": h, "qh": qh, "ctxraw": ctxraw, "rb": rb}

            # ---------------- output projection ----------------
            # (ot, 0) units first: they only need q columns 0-1023, already
            # normalized. The final sub-phase's norm is emitted a few units
            # in so it doesn't head-of-line-block their DVE evictions while
            # its broadcast rb is still in flight.
            norm_pending()
            order = [(ot, 0) for ot in range(BT)] + [(ot, 1) for ot in range(BT)]
            for idx, (ot, bh) in enumerate(order):
                emit_cross(ot, bh, idx, use_b=(idx % 2 == 1))

    nc.compile()
    return nc


def _get_compiled():
    global _compiled
    if _compiled is None:
        _compiled = _build()
    return _compiled


def _numpy_reference(hidden_states, attention_mask, Wq, Wk, Wv, Wo, scale_param):
    hs = np.asarray(hidden_states, np.float64)
    q = (hs @ np.asarray(Wq, np.float64).T).reshape(B, NH, HD).transpose(1, 0, 2)
    k = (hs @ np.asarray(Wk, np.float64).T).reshape(B, NH, HD).transpose(1, 0, 2)
    v = (hs @ np.asarray(Wv, np.float64).T).reshape(B, NH, HD).transpose(1, 0, 2)
    scores = np.einsum("hqd,hkd->hqk", q, k) / (HD ** 0.5)
    eye = np.eye(B, dtype=bool)
    scores = np.where(eye[None, :, :], -np.inf, scores)
    mask = np.asarray(attention_mask, bool)
    scores = np.where(mask[None, None, :], scores, -np.inf)
    m = scores.max(axis=-1, keepdims=True)
    m = np.where(np.isfinite(m), m, 0.0)
    e = np.exp(scores - m)
    s = e.sum(axis=-1, keepdims=True)
    attn = np.where(s > 0, e / np.maximum(s, 1e-300), 0.0)
    ctx = np.einsum("hqk,hkd->hqd", attn, v)
    ctx = ctx.transpose(1, 0, 2).reshape(B, H)
    cross = ctx @ np.asarray(Wo, np.float64).T
    scale = 1.0 / (1.0 + np.exp(-float(np.asarray(scale_param).reshape(-1)[0])))
    return (hs + scale * cross).astype(np.float32)


LAST_RESULTS = None


def kernel(hidden_states, attention_mask, Wq, Wk, Wv, Wo, scale_param):
    hs = np.asarray(hidden_states, np.float32)
    mask = np.asarray(attention_mask, bool)
    if not mask.all():
        return _numpy_reference(hidden_states, mask, Wq, Wk, Wv, Wo, scale_param)

    from concourse import bass_utils

    nc = _get_compiled()

    def pack(a):
        # [T*128, W] -> [128, T*W]: row p holds tile-t's row p for every t,
        # matching the SBUF destination layout exactly (contiguous DMA rows)
        t = a.shape[0] // P
        return np.ascontiguousarray(
            a.reshape(t, P, a.shape[1]).transpose(1, 0, 2).reshape(P, t * a.shape[1])
        )

    from concourse import mybir as _mybir
    _F8 = _mybir.dt.np(_mybir.dt.float8e4)

    hT = pack(hs.T.astype(_F8))
    antiI = (1.0 - np.eye(P, dtype=np.float32)).astype(_BF16)
    Wq = np.asarray(Wq, np.float32)
    Wk = np.asarray(Wk, np.float32)
    Wv = np.asarray(Wv, np.float32)
    Wo = np.asarray(Wo, np.float32)

    in_maps = []
    for c in range(NCORES):
        rs = slice(c * E, (c + 1) * E)
        in_maps.append({
            "hT": hT,
            "wqT": pack((Wq[rs, :].T * np.float32(16.0)).astype(_F8)),
            "wkT": pack((Wk[rs, :].T * np.float32(16.0)).astype(_F8)),
            "wvT": pack((Wv[rs, :].T * np.float32(16.0)).astype(_F8)),
            "woT": pack((Wo[:, rs].T * np.float32(16.0)).astype(_F8)),
            "antiI": antiI,
        })

    import os
    res = bass_utils.run_bass_kernel_spmd(
        nc, in_maps, core_ids=list(range(NCORES)),
        trace=bool(os.environ.get("KERNEL_TRACE")),
    )
    global LAST_RESULTS
    LAST_RESULTS = res

    cross = np.zeros((H, B), np.float32)
    for r in res.results:
        cross += np.asarray(r["out"], np.float32)
    # device emits crossT[o, b] at 1024x natural scale (ctxT 64x, wo 16x)
    cross = cross.T * np.float32(1.0 / 1024.0)
    scale = np.float32(1.0 / (1.0 + np.exp(-float(np.asarray(scale_param).reshape(-1)[0]))))
    return (hs + scale * cross).astype(np.float32)

